# revision 15
# baseline (speedup 1.0000x reference)
"""Trainium2 Bass kernel for flax MultiHeadDotProductAttention.

Shapes (hardcoded): B=4, Q=K=1500, D=1024, H=16, HD=64.
Sharding: 8 cores = 4 batches x 2 head-groups (8 heads each).
Each core computes its batch's attention output for its 8 heads plus the
output projection restricted to those heads; the host sums the two
head-group partials per batch and adds bo.

Dataflow per core (all layouts chosen so no on-device transposes are
needed; host passes x pre-transposed):
  qT/kT [hhd, seq] and v [seq, hhd] via projection matmuls (bf16);
  S^T[k,q] = kT.T-slices @ qT (K=64, row-packed 2 heads per PE slot —
  the two head matmuls sit on different PE row-groups and overlap);
  P^T = exp(S^T/8) on ScalarE (psum->sbuf, bf16);
  attn_outT += v_tile.T @ P^T and denominators via a ones column in v
  (row 64 of the same psum);
  normalization via a selector matmul broadcast + one full-width
  approximate reciprocal; out-projection consumes the normalized
  [hhd, q] tiles as stationary operands -> natural [q, d] output tiles
  DMA'd straight to HBM.

Schedule: ScalarE's exp stream is the critical path (~150us of work
that only this engine can do), so the kernel starts it as early as
possible and keeps it fed:
  - phase 1 projects ONLY kT block 0 / q-chunk 0 and qT block 0 /
    q-chunk 0 (first exp at ~15us instead of ~50us); every other
    projection group (kT blocks, qT blocks, v tiles) is emitted
    interleaved into the attention k-loops ahead of its first use;
  - the out-projection of chunk c is emitted interleaved into chunk
    c+1's first pair so ScalarE never waits on it (its psum->sbuf
    copies ride on the otherwise-idle GpSimd engine);
  - x DMAs are split by column so the first projection group only
    waits on the columns it reads.
"""

import sys

sys.path.insert(0, "/opt/trn_rl_repo")

import numpy as np  # noqa: E402
import ml_dtypes  # noqa: E402
import concourse.bacc as bacc  # noqa: E402
import concourse.mybir as mybir  # noqa: E402
import concourse.tile as tile  # noqa: E402
from concourse.bass_utils import run_bass_kernel_spmd  # noqa: E402

F32 = mybir.dt.float32
F32R = mybir.dt.float32r
BF16 = mybir.dt.bfloat16
AF = mybir.ActivationFunctionType

B, SEQ, D, H, HD = 4, 1500, 1024, 16, 64
HG = 8                      # heads per group
HHD = HG * HD               # 512
DCH = D // 128              # 8 d-chunks
HB = HHD // 128             # 4 hhd blocks (2 heads each)
NPAIR = HB                  # 4 head pairs per group
QC = [(0, 512), (512, 512), (1024, 476)]          # q chunks
KT = [(i * 128, min(128, SEQ - i * 128)) for i in range((SEQ + 127) // 128)]
NKT = len(KT)               # 12 (last tile 92 rows)


def _build(with_bias):
    nc = bacc.Bacc("TRN2", target_bir_lowering=False, debug=False, num_devices=8)

    xqT = nc.declare_dram_parameter("xqT", [D, SEQ], BF16, isOutput=False)
    xkvT = nc.declare_dram_parameter("xkvT", [D, SEQ], BF16, isOutput=False)
    wq_d = nc.declare_dram_parameter("wq", [D, HHD], BF16, isOutput=False)
    wk_d = nc.declare_dram_parameter("wk", [D, HHD], BF16, isOutput=False)
    wv_d = nc.declare_dram_parameter("wv", [D, HHD], BF16, isOutput=False)
    wo_d = nc.declare_dram_parameter("wo", [HHD, D], BF16, isOutput=False)
    bq_d = nc.declare_dram_parameter("bq", [1, HHD], BF16, isOutput=False)
    bk_d = nc.declare_dram_parameter("bk", [1, HHD], BF16, isOutput=False)
    bv_d = nc.declare_dram_parameter("bv", [1, HHD], BF16, isOutput=False)
    sel_d = nc.declare_dram_parameter("sel", [128, 64], F32, isOutput=False)
    zr_d = nc.declare_dram_parameter("zr", [128, 512], F32, isOutput=False)
    out_d = nc.declare_dram_parameter("out", [SEQ, D], F32, isOutput=True)

    with tile.TileContext(nc) as tc:
        from contextlib import ExitStack

        with ExitStack() as ctx:
            ctx.enter_context(nc.allow_low_precision(
                reason="bf16 matmul operands; psum accumulation is fp32"
            ))
            const = ctx.enter_context(tc.tile_pool(name="const", bufs=1))
            ones_r = const.tile([1, 512], BF16, tag="ones")
            nc.vector.memset(ones_r[:], 1.0)
            sel_sb = const.tile([128, 64], F32R, tag="sel")
            ds_e = const.tile([128, 512], F32R, tag="dse")
            ds_o = const.tile([128, 512], F32R, tag="dso")
            bq_sb = const.tile([1, HHD], BF16, tag="bq")
            bk_sb = const.tile([1, HHD], BF16, tag="bk")
            bv_sb = const.tile([1, HHD], BF16, tag="bv")

            def load_consts():
                # deferred: not needed until the first normalize, so these
                # DMAs must not delay the phase-1 weight/input loads
                nc.sync.dma_start(sel_sb[:], sel_d[:].bitcast(F32R))
                nc.sync.dma_start(ds_e[:], zr_d[:].bitcast(F32R))
                nc.sync.dma_start(ds_o[:], zr_d[:].bitcast(F32R))
                nc.sync.dma_start(bq_sb[:], bq_d[:])
                nc.sync.dma_start(bk_sb[:], bk_d[:])
                nc.sync.dma_start(bv_sb[:], bv_d[:])

            # persistent activations for the attention phase
            qT_b = []
            for i in range(HB):
                qT_b.append(const.tile([128, SEQ], BF16, tag=f"qT{i}",
                                       name=f"qT{i}"))  # [hhd%128, q] per block
            kT = const.tile([128, HB, SEQ], BF16, tag="kT")
            # v: one tile per k-tile ([k%128, head, hd|1]); the 65th column
            # is ones so attn@V also accumulates the softmax denominator
            # into row 64
            v_t = []
            for kt in range(NKT):
                vt = const.tile([128, HG, 65], BF16, tag=f"v{kt}", name=f"v{kt}")
                nc.vector.memset(vt[:, :, 64:65], 1.0)
                v_t.append(vt)

            wpool = ctx.enter_context(tc.tile_pool(name="w", bufs=2))

            def load_w(dram):
                # per-d-chunk tiles: the first projection matmul only waits
                # on its own 128-row slice of the weight, not the whole DMA
                ts = []
                d3 = dram.rearrange("(c p) n -> c p n", p=128)
                for c in range(D // 128):
                    t = wpool.tile([128, HHD], BF16, tag="wc", bufs=24,
                                   name=f"w{c}")
                    nc.sync.dma_start(t[:], d3[c])
                    ts.append(t)
                return ts

            xpool = ctx.enter_context(tc.tile_pool(name="x", bufs=16))

            def load_x(dram, split=False):
                # split=True: two DMAs per chunk (cols 0:512 first) so the
                # first q-chunk projection starts before the tail columns land
                xs = []
                for c in range(DCH):
                    t = xpool.tile([128, SEQ], BF16, tag="xc", bufs=16)
                    if split:
                        nc.sync.dma_start(
                            t[:, 0:512], dram[c * 128:(c + 1) * 128, 0:512]
                        )
                        nc.sync.dma_start(
                            t[:, 512:SEQ], dram[c * 128:(c + 1) * 128, 512:SEQ]
                        )
                    else:
                        nc.sync.dma_start(
                            t[:], dram[c * 128:(c + 1) * 128, :]
                        )
                    xs.append(t)
                return xs

            # ---------------- phase 1: input loads + first projections ----
            # k-proj weights+inputs first (kT block 0 gates the first exp)
            wk_d3 = wk_d.rearrange("(c p) n -> c p n", p=128)
            wk_sb, xkv = [], []
            for c in range(DCH):
                t = wpool.tile([128, HHD], BF16, tag="wc", bufs=24,
                               name=f"wk{c}")
                nc.sync.dma_start(t[:], wk_d3[c])
                wk_sb.append(t)
                tx = xpool.tile([128, SEQ], BF16, tag="xc", bufs=16)
                nc.sync.dma_start(tx[:, 0:512], xkvT[c * 128:(c + 1) * 128, 0:512])
                xkv.append(tx)
            for c in range(DCH):
                nc.sync.dma_start(
                    xkv[c][:, 512:SEQ], xkvT[c * 128:(c + 1) * 128, 512:SEQ]
                )
            wq_sb = load_w(wq_d)
            xq = load_x(xqT, split=True)
            load_consts()
            wv_sb = load_w(wv_d)
            wo_sb = wpool.tile([128, HB, D], BF16, tag="w", bufs=2)
            nc.sync.dma_start(
                wo_sb[:], wo_d.rearrange("(c p) n -> p c n", p=128)
            )

            st_ps = ctx.enter_context(tc.tile_pool(name="stps", bufs=2, space="PSUM"))
            at_ps = ctx.enter_context(tc.tile_pool(name="atps", bufs=3, space="PSUM"))
            rbo_ps = ctx.enter_context(tc.tile_pool(name="rbops", bufs=1, space="PSUM"))
            p_pool = ctx.enter_context(tc.tile_pool(name="p", bufs=20))
            an_pool = ctx.enter_context(tc.tile_pool(name="an", bufs=8))
            small = ctx.enter_context(tc.tile_pool(name="small", bufs=4))

            def projT_group(dst2d, w_sb, b_sb, xs, hb, qci,
                            psum_pool, ptag, pbufs):
                qo2, cw2 = QC[qci]
                ps = psum_pool.tile([128, 512], F32, tag=ptag, bufs=pbufs,
                                    name=f"tps{hb}_{qci}")
                for c in range(DCH):
                    nc.tensor.matmul(
                        ps[:, :cw2],
                        w_sb[c][:, hb * 128:(hb + 1) * 128],
                        xs[c][:, qo2:qo2 + cw2],
                        start=(c == 0), stop=(not with_bias and c == DCH - 1),
                    )
                if with_bias:
                    nc.tensor.matmul(
                        ps[:, :cw2],
                        b_sb[0:1, hb * 128:(hb + 1) * 128],
                        ones_r[0:1, :cw2],
                        start=False, stop=True,
                    )
                nc.vector.tensor_copy(dst2d[:, qo2:qo2 + cw2], ps[:, :cw2])

            def proj_v_group(kt, psum_pool, ptag, pbufs):
                ko, kh = KT[kt]
                ps = psum_pool.tile([128, 512], F32, tag=ptag, bufs=pbufs,
                                    name=f"vps{kt}")
                for c in range(DCH):
                    nc.tensor.matmul(
                        ps[:kh, :],
                        xkv[c][:, ko:ko + kh],
                        wv_sb[c][:, :],
                        start=(c == 0), stop=(not with_bias and c == DCH - 1),
                    )
                if with_bias:
                    nc.tensor.matmul(
                        ps[:kh, :],
                        ones_r[0:1, :kh],
                        bv_sb[0:1, :],
                        start=False, stop=True,
                    )
                nc.vector.tensor_copy(
                    v_t[kt][:kh, :, 0:64],
                    ps[:kh, :].rearrange("p (h c) -> p h c", c=64),
                )

            # interleaved projection groups alternate between the at_ps
            # spare slot and the rbo bank so group g+1's matmuls overlap
            # group g's psum->sbuf copy
            galt = [0]

            def _alt_pool():
                galt[0] ^= 1
                return (at_ps, "attn", 3) if galt[0] else (rbo_ps, "rbo", 1)

            def kt_group(hb, qci):
                def emit():
                    pool, tag, bufs = _alt_pool()
                    projT_group(kT[:, hb, :], wk_sb, bk_sb, xkv,
                                hb, qci, pool, tag, bufs)
                return emit

            def qt_group(hb, qci):
                def emit():
                    pool, tag, bufs = _alt_pool()
                    projT_group(qT_b[hb], wq_sb, bq_sb, xq,
                                hb, qci, pool, tag, bufs)
                return emit

            # the ONLY projections emitted before the attention loop: the
            # two groups the first S^T step needs
            projT_group(kT[:, 0, :], wk_sb, bk_sb, xkv, 0, 0, at_ps, "attn", 3)
            projT_group(qT_b[0], wq_sb, bq_sb, xq, 0, 0, at_ps, "attn", 3)

            # remaining projection groups, scheduled into chunk-0 k-loops
            # just ahead of their first consumer (kT block j chunk c is
            # needed by pair j's steps 4c..4c+3; qT block j chunks by pair
            # j of each q-chunk)
            sched0 = {
                (0, 2): [kt_group(0, 1)], (0, 5): [qt_group(1, 0)],
                (0, 6): [kt_group(0, 2)], (0, 7): [kt_group(1, 0)],
                (0, 9): [kt_group(1, 1)],
                (1, 3): [kt_group(1, 2)], (1, 5): [qt_group(2, 0)],
                (1, 6): [kt_group(2, 0)], (1, 8): [kt_group(2, 1)],
                (2, 3): [kt_group(2, 2)], (2, 5): [qt_group(3, 0)],
                (2, 6): [kt_group(3, 0)], (2, 8): [kt_group(3, 1)],
                (3, 3): [kt_group(3, 2)], (3, 4): [qt_group(0, 1)],
            }
            # qT groups for chunks 1/2 ride in those chunks (their pair-0
            # steps are otherwise light); keyed by (chunk, pair, kt)
            sched_c = {
                (1, 0, 9): [qt_group(1, 1)], (1, 1, 5): [qt_group(2, 1)],
                (1, 2, 5): [qt_group(3, 1)], (1, 2, 9): [qt_group(0, 2)],
                (2, 0, 9): [qt_group(1, 2)], (2, 1, 5): [qt_group(2, 2)],
                (2, 2, 5): [qt_group(3, 2)],
            }

            # ---------------- attention + out-proj ----------------
            def make_op_group(an_list, qo_c, cw_c, s, dc, last):
                sw = min(128, cw_c - s * 128)

                def emit():
                    alt = last and (s * 2 + dc) % 2 == 1
                    op = (st_ps if alt else at_ps).tile(
                        [128, 512], F32, tag=("st" if alt else "attn"),
                        bufs=(2 if alt else 3), name=f"op{int(alt)}")
                    for jj in range(NPAIR):
                        nc.tensor.matmul(
                            op[:sw, :],
                            an_list[jj][:, s * 128:s * 128 + sw],
                            wo_sb[:, jj, dc * 512:(dc + 1) * 512],
                            start=(jj == 0), stop=(jj == NPAIR - 1),
                        )
                    osb = small.tile([128, 512], F32, tag="os", bufs=3)
                    if last:
                        # ScalarE is idle after the last exp; keep DVE
                        # free for the trailing normalize ops
                        nc.scalar.copy(osb[:sw, :], op[:sw, :])
                    else:
                        nc.vector.tensor_copy(osb[:sw, :], op[:sw, :])
                    nc.sync.dma_start(
                        out_d[qo_c + s * 128:qo_c + s * 128 + sw,
                              dc * 512:(dc + 1) * 512],
                        osb[:sw, :],
                    )
                return emit

            def attn_v(pe_b, po_b, j, cw, kc, psrc):
                ko, kh = KT[kc]
                nc.tensor.matmul(
                    pe_b[0:65, :cw],
                    v_t[kc][0:kh, 2 * j, :],
                    psrc[0:kh, 0, :cw],
                    start=(kc == 0), stop=(kc == NKT - 1),
                )
                nc.tensor.matmul(
                    po_b[0:65, :cw],
                    v_t[kc][0:kh, 2 * j + 1, :],
                    psrc[0:kh, 1, :cw],
                    start=(kc == 0), stop=(kc == NKT - 1),
                )

            def normalize(pe_b, po_b, cw, anorms):
                # normalize: den row -> zeroed staging tile, selector
                # matmul broadcasts it to 64 partitions, approx recip,
                # multiply into the normalized attn tile; the odd head is
                # recombined into partitions 64..127 via an SBUF DMA
                an = an_pool.tile([128, 512], BF16, tag="an", bufs=8)
                nc.vector.tensor_copy(ds_e[64:65, :cw], pe_b[64:65, :cw])
                rb_eps = rbo_ps.tile([128, 512], F32, tag="rbo", bufs=1,
                                     name="rb_e")
                nc.tensor.matmul(
                    rb_eps[0:64, :cw], sel_sb[:, :], ds_e[:, :cw],
                    start=True, stop=True,
                )
                rb_esb = small.tile([64, 512], F32, tag="rb", bufs=2,
                                    name="rb_esb")
                nc.vector.reciprocal_approx_fast(rb_esb[:, :cw], rb_eps[0:64, :cw])
                nc.vector.tensor_mul(
                    an[0:64, :cw], pe_b[0:64, :cw], rb_esb[:, :cw]
                )
                nc.vector.tensor_copy(ds_o[64:65, :cw], po_b[64:65, :cw])
                rb_ops = rbo_ps.tile([128, 512], F32, tag="rbo", bufs=1,
                                     name="rb_o")
                nc.tensor.matmul(
                    rb_ops[0:64, :cw], sel_sb[:, :], ds_o[:, :cw],
                    start=True, stop=True,
                )
                rb_osb = small.tile([64, 512], F32, tag="rb", bufs=2,
                                    name="rb_osb")
                nc.vector.reciprocal_approx_fast(rb_osb[:, :cw], rb_ops[0:64, :cw])
                antmp = small.tile([64, 512], BF16, tag="antmp", bufs=2)
                nc.vector.tensor_mul(
                    antmp[:, :cw], po_b[0:64, :cw], rb_osb[:, :cw]
                )
                nc.sync.dma_start(an[64:128, :cw], antmp[:, :cw])
                anorms.append(an)

            # flattened pipeline across pairs and chunks: the trailing
            # attn@V of pair p and its normalize are emitted inside pair
            # p+1's k-loop (at kt=0), so ScalarE's exp stream never breaks
            # at a pair boundary
            op_queue = []
            prev = None          # (pe_b, po_b, j, cw, last P tile, anorms)
            anorms = []
            chunk_anorms = {}
            for ci, (qo, cw) in enumerate(QC):
                chunk_anorms[ci] = anorms
                for j in range(NPAIR):
                    pe_b = at_ps.tile([128, 512], F32, tag="attn", bufs=3,
                                      name=f"pe_{j}")
                    po_b = at_ps.tile([128, 512], F32, tag="attn", bufs=3,
                                      name=f"po_{j}")
                    pend = None
                    for kt in range(NKT):
                        ko, kh = KT[kt]
                        st = st_ps.tile([128, 2, 512], F32, tag="st", bufs=2)
                        nc.tensor.matmul(
                            st[:kh, 0, :cw],
                            kT[0:64, j, ko:ko + kh],
                            qT_b[j][0:64, qo:qo + cw],
                            start=True, stop=True,
                        )
                        nc.tensor.matmul(
                            st[:kh, 1, :cw],
                            kT[64:128, j, ko:ko + kh],
                            qT_b[j][64:128, qo:qo + cw],
                            start=True, stop=True,
                        )
                        p = p_pool.tile([128, 2, 512], BF16, tag="p", bufs=20)
                        nc.scalar.activation(
                            p[:kh, :, :cw], st[:kh, :, :cw], AF.Exp,
                            scale=0.125,
                        )
                        # interleaved fill work goes AFTER the exp emission so
                        # the exp never waits behind a projection group on the
                        # in-order PE
                        if ci == 0:
                            if j == 0:
                                pool, tag, bufs = _alt_pool()
                                proj_v_group(kt, pool, tag, bufs)
                            for fn in sched0.get((j, kt), ()):
                                fn()
                        else:
                            for fn in sched_c.get((ci, j, kt), ()):
                                fn()
                            if op_queue and kt % 2 == 1:
                                # out-projection of the previous chunk rides
                                # in this chunk's early steps, every other
                                # step to keep ScalarE fed
                                op_queue.pop(0)()
                        if kt > 0:
                            attn_v(pe_b, po_b, j, cw, kt - 1, pend)
                        elif prev is not None:
                            # finish the previous pair under this pair's
                            # first exp, then normalize it
                            ppe, ppo, pj, pcw, pp, pan = prev
                            attn_v(ppe, ppo, pj, pcw, NKT - 1, pp)
                            normalize(ppe, ppo, pcw, pan)
                        pend = p
                    prev = (pe_b, po_b, j, cw, pend, anorms)

                # out-projection groups for this q chunk are deferred into
                # the next chunk's early steps (the last chunk drains after
                # the loop)
                nsub = (cw + 127) // 128
                last_chunk = ci == len(QC) - 1
                anorms_c, anorms = anorms, []
                for s in range(nsub):
                    for dc in range(2):
                        op_queue.append(
                            make_op_group(anorms_c, qo, cw, s, dc, last_chunk)
                        )

            # drain: last pair + its normalize, then the last chunk's
            # out-projection
            ppe, ppo, pj, pcw, pp, pan = prev
            attn_v(ppe, ppo, pj, pcw, NKT - 1, pp)
            normalize(ppe, ppo, pcw, pan)
            for fn in op_queue:
                fn()

    nc.compile()
    return nc


_NC = {}


def _get_nc(with_bias=False):
    if with_bias not in _NC:
        _NC[with_bias] = _build(with_bias)
    return _NC[with_bias]


def _sel_const():
    # broadcast matrix: den staging row 64 -> all 64 output partitions
    sel = np.zeros((128, 64), np.float32)
    sel[64, :] = 1.0
    return sel


def _shard_inputs(inputs_q, inputs_kv, Wq, bq, Wk, bk, Wv, bv, Wo, bo):
    ndt = ml_dtypes.bfloat16
    sel = _sel_const()
    zr = np.zeros((128, 512), np.float32)
    in_maps = []
    for b in range(B):
        xqT = np.ascontiguousarray(inputs_q[b].T).astype(ndt)
        xkvT = np.ascontiguousarray(inputs_kv[b].T).astype(ndt)
        for g in range(2):
            hs = slice(g * HG, (g + 1) * HG)
            in_maps.append({
                "xqT": xqT,
                "xkvT": xkvT,
                "wq": np.ascontiguousarray(Wq[:, hs, :].reshape(D, HHD)).astype(ndt),
                "wk": np.ascontiguousarray(Wk[:, hs, :].reshape(D, HHD)).astype(ndt),
                "wv": np.ascontiguousarray(Wv[:, hs, :].reshape(D, HHD)).astype(ndt),
                "wo": np.ascontiguousarray(Wo[hs].reshape(HHD, D)).astype(ndt),
                "bq": np.ascontiguousarray(bq[hs].reshape(1, HHD)).astype(ndt),
                "bk": np.ascontiguousarray(bk[hs].reshape(1, HHD)).astype(ndt),
                "bv": np.ascontiguousarray(bv[hs].reshape(1, HHD)).astype(ndt),
                "sel": sel,
                "zr": zr,
            })
    return in_maps


def _run(inputs, trace=False, trace_kwargs=None):
    inputs = {k: np.asarray(v) for k, v in inputs.items()}
    with_bias = bool(
        np.any(inputs["bq"]) or np.any(inputs["bk"]) or np.any(inputs["bv"])
    )
    nc = _get_nc(with_bias)
    in_maps = _shard_inputs(**inputs)
    res = run_bass_kernel_spmd(
        nc, in_maps, core_ids=list(range(2 * B)), trace=trace,
        **(trace_kwargs or {}),
    )
    bo = np.asarray(inputs["bo"], np.float32)
    out = np.empty((B, SEQ, D), np.float32)
    for b in range(B):
        out[b] = res.results[2 * b]["out"] + res.results[2 * b + 1]["out"] + bo
    return out, res


def kernel(**inputs):
    out, _ = _run(inputs, trace=False)
    return out
